# revision 5
# baseline (speedup 1.0000x reference)
"""GCN layer on 8 Trainium2 NeuronCores.

Computation (N=8192 nodes, IN=OUT=512):
    deg    = adj.sum(1)
    dis    = (deg + 1e-8) ** -0.5
    a_norm = dis[:, None] * adj * dis[None, :]
    out    = (a_norm @ x) @ W.T + b

Distribution: 1D row shard. Core c owns rows R_c = [1024c, 1024(c+1)).
The host hands each core its adj shard PRE-TRANSPOSED and cast to fp16
(adjT[k, i] = adj[row i of shard, k]) so every PE matmul sees the
contraction dim on partitions with fully contiguous DMA; x (fp16) /
W^T (fp16) / b / dis are replicated.

The degree vector (an O(N^2) -> O(N) reduction, 0.09% of the FLOPs) is
computed host-side during input sharding and shipped as the tiny `dis`
inputs. This removes the device-side AllGather that previously sat
between the deg pass and the main matmul: profiling showed the
collective costing ~117us of pure PE idle (a NEFF-start barrier
absorbing inter-core launch skew blocked the CC stream, then a 50us
AllGather for 4KB/rank). With no cross-core dependency every core runs
start-to-finish independently and launch skew no longer serializes.

Per-core device program (SPMD, identical on all cores):
  0) warmup: a few junk matmuls lift the PE HAM clock-gate (1.2 ->
     2.4 GHz) while the first adj/x chunks stream in.
  A) stream the 16MB fp16 adjT shard into SBUF (ascending chunk sizes
     so compute starts ~1us in); stream x in chunks on a second DMA
     queue; scale x rows by dis (per-partition scalars, DVE) and feed
     the big matmul G^T[f, i] = sum_k y[k, f] adjT[k, i] as tiles
     arrive, accumulating across all 64 k-tiles in all 8 PSUM banks.
  B) evict G^T with the row scale (dis broadcast along free dim) to
     fp16, then out = G @ W^T + b (fp16 matmuls, f32 bias add) and DMA
     rows out.
"""

import os
import sys

import numpy as np

for _p in ("/opt/trn_rl_repo",):
    if os.path.isdir(_p) and _p not in sys.path:
        sys.path.append(_p)

import concourse.bass as bass  # noqa: E402
import concourse.mybir as mybir  # noqa: E402
import concourse.tile as tile  # noqa: E402
from concourse import bacc  # noqa: E402
from concourse.bass_utils import run_bass_kernel_spmd  # noqa: E402

N, IN, OUT = 8192, 512, 512
N_CORES = 8
R = N // N_CORES  # rows per core = 1024
KT = N // 128  # k-tiles = 64
EPS = 1e-08

F32 = mybir.dt.float32
F16 = mybir.dt.float16

# ascending chunk sizes (in 128-row k-tiles): tiny first chunks get the
# matmul stream started ~1us in, big tail chunks keep DMA efficiency.
CHUNKS = [1, 1, 2, 4] + [8] * 7
assert sum(CHUNKS) == KT


def _build():
    nc = bacc.Bacc(
        "TRN2", target_bir_lowering=False, debug=False, num_devices=N_CORES
    )

    # adj/x are host-prearranged partition-major ([p, u, ...]) so every
    # DMA line is a long contiguous read (16KB / 8KB per partition per
    # 8-tile chunk). The natural [k, m] layout put only 1-2KB per line
    # and measured 60-125 GB/s on the x stream — starving the PE early.
    adjT_d = nc.dram_tensor("adjT", [128, KT * R], F16, kind="ExternalInput").ap()
    x_d = nc.dram_tensor("x", [128, KT * IN], F16, kind="ExternalInput").ap()
    wT_d = nc.dram_tensor("wT", [IN, OUT], F16, kind="ExternalInput").ap()
    b_d = nc.dram_tensor("b", [1, OUT], F32, kind="ExternalInput").ap()
    disk_d = nc.dram_tensor("disk", [128, KT], F32, kind="ExternalInput").ap()
    disr_d = nc.dram_tensor("disr", [1, R], F32, kind="ExternalInput").ap()
    out_d = nc.dram_tensor("out", [R, OUT], F32, kind="ExternalOutput").ap()

    adjT_v = adjT_d.rearrange("p (u m) -> p u m", m=R)  # [128, 64, 1024]
    x_v = x_d.rearrange("p (u f) -> p u f", f=IN)  # [128, 64, 512]
    out_v = out_d.rearrange("(i p) o -> p i o", p=128)  # [128, 8, 512]

    with tile.TileContext(nc) as tc:
        with (
            tc.tile_pool(name="cpool", bufs=1) as cpool,
            tc.tile_pool(name="ypool", bufs=2) as ypool,
            tc.tile_pool(name="opool", bufs=2) as opool,
            tc.tile_pool(name="ps", bufs=8, space="PSUM") as ps,
        ):
            # ---- small loads: dis scalars first (gate the x scaling) ----
            disk_sb = cpool.tile([128, KT], F32)
            nc.scalar.dma_start(disk_sb[:], disk_d[:])
            # tail-only constants on the gpsimd queue so they never delay
            # the x stream on the scalar queue.
            wT_sb = cpool.tile([128, 4, 512], F16)
            nc.gpsimd.dma_start(wT_sb[:], wT_d.rearrange("(t p) o -> p t o", p=128))
            bb = cpool.tile([128, 512], F32)
            nc.gpsimd.dma_start(bb[:], b_d.to_broadcast((128, 512)))
            disr_bc = cpool.tile([128, R], F32)
            nc.gpsimd.dma_start(disr_bc[:], disr_d.to_broadcast((128, R)))

            adj = cpool.tile([128, KT, 1024], F16)  # whole shard, resident
            gps = [
                ps.tile([128, 512], F32, tag="ps", name=f"gps{i}") for i in range(8)
            ]

            # ---- PE warmup: junk matmuls while the first chunks stream in.
            # HAM needs ~3.4us of busy to lift the 1.2GHz cold gate; these
            # overwrite gps[0] which the first real matmul clears anyway.
            junk = cpool.tile([128, 128], F16)
            nc.vector.memset(junk[:], 0.0)
            for _ in range(28):
                nc.tensor.matmul(
                    gps[0][:, :128], junk[:], junk[:], start=True, stop=True
                )

            # ---- main stream: adj on sync queue, x on scalar queue ----
            u0 = 0
            for csz in CHUNKS:
                nc.sync.dma_start(
                    adj[:, u0 : u0 + csz, :], adjT_v[:, u0 : u0 + csz, :]
                )
                yc = ypool.tile([128, 8, 512], F16, tag="yc", name="yc")
                nc.scalar.dma_start(
                    yc[:, :csz, :], x_v[:, u0 : u0 + csz, :]
                )
                for t in range(csz):
                    u = u0 + t
                    nc.vector.tensor_scalar_mul(
                        yc[:, t, :], yc[:, t, :], disk_sb[:, u : u + 1]
                    )
                    for ft in range(4):
                        lhs = yc[:, t, 128 * ft : 128 * (ft + 1)]
                        for ih in range(2):
                            nc.tensor.matmul(
                                gps[ft * 2 + ih][:],
                                lhs,
                                adj[:, u, 512 * ih : 512 * (ih + 1)],
                                start=(u == 0),
                                stop=(u == KT - 1),
                            )
                u0 += csz

            # ---- evict with row scaling (fp16), then out = G @ W^T + b.
            # ih-major so phase D on the first m-half starts while the
            # second half is still being evicted.
            gsb = cpool.tile([128, 4, 1024], F16)
            for ih in range(2):
                for ft in range(4):
                    nc.vector.tensor_mul(
                        gsb[:, ft, 512 * ih : 512 * (ih + 1)],
                        gps[ft * 2 + ih][:],
                        disr_bc[:, 512 * ih : 512 * (ih + 1)],
                    )
                for i in range(4 * ih, 4 * (ih + 1)):
                    op = ps.tile([128, 512], F32, tag="ps", name="op")
                    for ft in range(4):
                        nc.tensor.matmul(
                            op[:],
                            gsb[:, ft, 128 * i : 128 * (i + 1)],
                            wT_sb[:, ft, :],
                            start=(ft == 0),
                            stop=(ft == 3),
                        )
                    osb = opool.tile([128, 512], F32, tag="osb", name="osb")
                    nc.vector.tensor_add(osb[:], op[:], bb[:])
                    nc.sync.dma_start(out_v[:, i, :], osb[:])

    nc.compile()
    return nc


_NC_CACHE = None


def _get_nc():
    global _NC_CACHE
    if _NC_CACHE is None:
        _NC_CACHE = _build()
    return _NC_CACHE


def _make_in_maps(x, adj, W, b):
    x = np.asarray(x, dtype=np.float32)
    adj = np.asarray(adj, dtype=np.float32)
    W = np.asarray(W, dtype=np.float32)
    b = np.asarray(b, dtype=np.float32)

    deg = adj.sum(axis=1, dtype=np.float64)
    dis = ((deg + EPS) ** -0.5).astype(np.float32)  # [N]

    # partition-major: [k, ...] -> [p, u, ...] -> [128, u*...]
    x_bf = np.ascontiguousarray(
        x.astype(np.float16).reshape(KT, 128, IN).transpose(1, 0, 2).reshape(128, -1)
    )
    wT = np.ascontiguousarray(W.T.astype(np.float16))
    b2 = np.ascontiguousarray(b.reshape(1, OUT))
    disk = np.ascontiguousarray(dis.reshape(KT, 128).T)  # [128, 64]
    in_maps = []
    for c in range(N_CORES):
        shard = np.ascontiguousarray(
            adj[c * R : (c + 1) * R, :]
            .T.astype(np.float16)
            .reshape(KT, 128, R)
            .transpose(1, 0, 2)
            .reshape(128, -1)
        )
        disr = np.ascontiguousarray(dis[c * R : (c + 1) * R].reshape(1, R))
        in_maps.append(
            {
                "adjT": shard,
                "x": x_bf,
                "wT": wT,
                "b": b2,
                "disk": disk,
                "disr": disr,
            }
        )
    return in_maps


def run(x, adj, W, b, trace=False, tmpdir=None):
    nc = _get_nc()
    in_maps = _make_in_maps(x, adj, W, b)
    res = run_bass_kernel_spmd(
        nc, in_maps, list(range(N_CORES)), trace=trace, tmpdir=tmpdir
    )
    out = np.concatenate(
        [res.results[c]["out"] for c in range(N_CORES)], axis=0
    ).astype(np.float32)
    return out, res


def kernel(x, adj, W, b):
    out, _ = run(x, adj, W, b, trace=False)
    return out


# revision 7
# speedup vs baseline: 1.2859x; 1.2859x over previous
"""GCN layer on 8 Trainium2 NeuronCores.

Computation (N=8192 nodes, IN=OUT=512):
    deg    = adj.sum(1)
    dis    = (deg + 1e-8) ** -0.5
    a_norm = dis[:, None] * adj * dis[None, :]
    out    = (a_norm @ x) @ W.T + b

Distribution: 1D row shard. Core c owns rows R_c = [1024c, 1024(c+1)).
The host hands each core its adj shard PRE-TRANSPOSED and cast to fp16
(adjT[k, i] = adj[row i of shard, k]) so every PE matmul sees the
contraction dim on partitions with fully contiguous DMA; x (fp16) /
W^T (fp16) / b / dis are replicated.

The degree vector (an O(N^2) -> O(N) reduction, 0.09% of the FLOPs) is
computed host-side during input sharding and shipped as the tiny `dis`
inputs. This removes the device-side AllGather that previously sat
between the deg pass and the main matmul: profiling showed the
collective costing ~117us of pure PE idle (a NEFF-start barrier
absorbing inter-core launch skew blocked the CC stream, then a 50us
AllGather for 4KB/rank). With no cross-core dependency every core runs
start-to-finish independently and launch skew no longer serializes.

Per-core device program (SPMD, identical on all cores):
  0) warmup: a few junk matmuls lift the PE HAM clock-gate (1.2 ->
     2.4 GHz) while the first adj/x chunks stream in.
  A) stream the 16MB fp16 adjT shard into SBUF (ascending chunk sizes
     so compute starts ~1us in); stream x in chunks on a second DMA
     queue; scale x rows by dis (per-partition scalars, DVE) and feed
     the big matmul G^T[f, i] = sum_k y[k, f] adjT[k, i] as tiles
     arrive, accumulating across all 64 k-tiles in all 8 PSUM banks.
  B) evict G^T with the row scale (dis broadcast along free dim) to
     fp16, then out = G @ W^T + b (fp16 matmuls, f32 bias add) and DMA
     rows out.
"""

import os
import sys

import numpy as np

for _p in ("/opt/trn_rl_repo",):
    if os.path.isdir(_p) and _p not in sys.path:
        sys.path.append(_p)

import concourse.bass as bass  # noqa: E402
import concourse.mybir as mybir  # noqa: E402
import concourse.tile as tile  # noqa: E402
from concourse import bacc  # noqa: E402
from concourse.bass_utils import run_bass_kernel_spmd  # noqa: E402

N, IN, OUT = 8192, 512, 512
N_CORES = 8
R = N // N_CORES  # rows per core = 1024
KT = N // 128  # k-tiles = 64
EPS = 1e-08

F32 = mybir.dt.float32
F16 = mybir.dt.float16

# ascending chunk sizes (in 128-row k-tiles): tiny first chunks get the
# matmul stream started ~1us in, big tail chunks keep DMA efficiency.
CHUNKS = [1, 1, 2, 4] + [8] * 7
assert sum(CHUNKS) == KT
YBUFS = 4  # x-chunk ring depth: absorbs per-chunk DMA trigger latency


def _build():
    nc = bacc.Bacc(
        "TRN2", target_bir_lowering=False, debug=False, num_devices=N_CORES
    )

    # adj/x are host-prearranged partition-major ([p, u, ...]) so every
    # DMA line is a long contiguous read (16KB / 8KB per partition per
    # 8-tile chunk). The natural [k, m] layout put only 1-2KB per line
    # and measured 60-125 GB/s on the x stream — starving the PE early.
    adjT_d = nc.dram_tensor("adjT", [128, KT * R], F16, kind="ExternalInput").ap()
    x_d = nc.dram_tensor("x", [128, KT * IN], F16, kind="ExternalInput").ap()
    wT_d = nc.dram_tensor("wT", [IN, OUT], F16, kind="ExternalInput").ap()
    b_d = nc.dram_tensor("b", [1, OUT], F32, kind="ExternalInput").ap()
    disk_d = nc.dram_tensor("disk", [128, KT], F32, kind="ExternalInput").ap()
    disr_d = nc.dram_tensor("disr", [1, R], F32, kind="ExternalInput").ap()
    out_d = nc.dram_tensor("out", [R, OUT], F32, kind="ExternalOutput").ap()

    adjT_v = adjT_d.rearrange("p (u m) -> p u m", m=R)  # [128, 64, 1024]
    x_v = x_d.rearrange("p (u f) -> p u f", f=IN)  # [128, 64, 512]
    out_v = out_d.rearrange("(i p) o -> p i o", p=128)  # [128, 8, 512]

    with tile.TileContext(nc) as tc:
        with (
            tc.tile_pool(name="cpool", bufs=1) as cpool,
            tc.tile_pool(name="ypool", bufs=YBUFS) as ypool,
            tc.tile_pool(name="opool", bufs=2) as opool,
            tc.tile_pool(name="ps", bufs=8, space="PSUM") as ps,
        ):
            # ---- small loads: dis scalars first (gate the x scaling) ----
            disk_sb = cpool.tile([128, KT], F32)
            nc.scalar.dma_start(disk_sb[:], disk_d[:])
            # tail-only constants on the gpsimd queue so they never delay
            # the x stream on the scalar queue.
            wT_sb = cpool.tile([128, 4, 512], F16)
            nc.gpsimd.dma_start(wT_sb[:], wT_d.rearrange("(t p) o -> p t o", p=128))
            bb = cpool.tile([128, 512], F32)
            nc.gpsimd.dma_start(bb[:], b_d.to_broadcast((128, 512)))
            disr_bc = cpool.tile([128, R], F32)
            nc.gpsimd.dma_start(disr_bc[:], disr_d.to_broadcast((128, R)))

            adj = cpool.tile([128, KT, 1024], F16)  # whole shard, resident
            gps = [
                ps.tile([128, 512], F32, tag="ps", name=f"gps{i}") for i in range(8)
            ]

            # ---- PE warmup: junk matmuls while the first chunks stream in.
            # HAM needs ~3.4us of busy to lift the 1.2GHz cold gate; these
            # overwrite gps[0] which the first real matmul clears anyway.
            junk = cpool.tile([128, 128], F16)
            nc.vector.memset(junk[:], 0.0)
            for _ in range(28):
                nc.tensor.matmul(
                    gps[0][:, :128], junk[:], junk[:], start=True, stop=True
                )

            # ---- main stream: adj on sync queue, x on scalar queue ----
            u0 = 0
            for csz in CHUNKS:
                nc.sync.dma_start(
                    adj[:, u0 : u0 + csz, :], adjT_v[:, u0 : u0 + csz, :]
                )
                yc = ypool.tile([128, 8, 512], F16, tag="yc", name="yc")
                nc.scalar.dma_start(
                    yc[:, :csz, :], x_v[:, u0 : u0 + csz, :]
                )
                for t in range(csz):
                    u = u0 + t
                    nc.vector.tensor_scalar_mul(
                        yc[:, t, :], yc[:, t, :], disk_sb[:, u : u + 1]
                    )
                    for ft in range(4):
                        lhs = yc[:, t, 128 * ft : 128 * (ft + 1)]
                        for ih in range(2):
                            nc.tensor.matmul(
                                gps[ft * 2 + ih][:],
                                lhs,
                                adj[:, u, 512 * ih : 512 * (ih + 1)],
                                start=(u == 0),
                                stop=(u == KT - 1),
                            )
                u0 += csz

            # ---- evict with row scaling (fp16), then out = G @ W^T + b.
            # ih-major so phase D on the first m-half starts while the
            # second half is still being evicted.
            gsb = cpool.tile([128, 4, 1024], F16)
            for ih in range(2):
                for ft in range(4):
                    nc.vector.tensor_mul(
                        gsb[:, ft, 512 * ih : 512 * (ih + 1)],
                        gps[ft * 2 + ih][:],
                        disr_bc[:, 512 * ih : 512 * (ih + 1)],
                    )
                for i in range(4 * ih, 4 * (ih + 1)):
                    op = ps.tile([128, 512], F32, tag="ps", name="op")
                    for ft in range(4):
                        nc.tensor.matmul(
                            op[:],
                            gsb[:, ft, 128 * i : 128 * (i + 1)],
                            wT_sb[:, ft, :],
                            start=(ft == 0),
                            stop=(ft == 3),
                        )
                    osb = opool.tile([128, 512], F32, tag="osb", name="osb")
                    nc.vector.tensor_add(osb[:], op[:], bb[:])
                    nc.sync.dma_start(out_v[:, i, :], osb[:])

    nc.compile()
    return nc


_NC_CACHE = None


def _get_nc():
    global _NC_CACHE
    if _NC_CACHE is None:
        _NC_CACHE = _build()
    return _NC_CACHE


def _make_in_maps(x, adj, W, b):
    x = np.asarray(x, dtype=np.float32)
    adj = np.asarray(adj, dtype=np.float32)
    W = np.asarray(W, dtype=np.float32)
    b = np.asarray(b, dtype=np.float32)

    deg = adj.sum(axis=1, dtype=np.float64)
    dis = ((deg + EPS) ** -0.5).astype(np.float32)  # [N]

    # partition-major: [k, ...] -> [p, u, ...] -> [128, u*...]
    x_bf = np.ascontiguousarray(
        x.astype(np.float16).reshape(KT, 128, IN).transpose(1, 0, 2).reshape(128, -1)
    )
    wT = np.ascontiguousarray(W.T.astype(np.float16))
    b2 = np.ascontiguousarray(b.reshape(1, OUT))
    disk = np.ascontiguousarray(dis.reshape(KT, 128).T)  # [128, 64]
    in_maps = []
    for c in range(N_CORES):
        shard = np.ascontiguousarray(
            adj[c * R : (c + 1) * R, :]
            .T.astype(np.float16)
            .reshape(KT, 128, R)
            .transpose(1, 0, 2)
            .reshape(128, -1)
        )
        disr = np.ascontiguousarray(dis[c * R : (c + 1) * R].reshape(1, R))
        in_maps.append(
            {
                "adjT": shard,
                "x": x_bf,
                "wT": wT,
                "b": b2,
                "disk": disk,
                "disr": disr,
            }
        )
    return in_maps


def run(x, adj, W, b, trace=False, tmpdir=None):
    nc = _get_nc()
    in_maps = _make_in_maps(x, adj, W, b)
    res = run_bass_kernel_spmd(
        nc, in_maps, list(range(N_CORES)), trace=trace, tmpdir=tmpdir
    )
    out = np.concatenate(
        [res.results[c]["out"] for c in range(N_CORES)], axis=0
    ).astype(np.float32)
    return out, res


def kernel(x, adj, W, b):
    out, _ = run(x, adj, W, b, trace=False)
    return out
